# revision 17
# baseline (speedup 1.0000x reference)
"""Trainium2 Bass kernel for the Gaussian-span multi-head self-attention module.

  span  = head_reshape(h @ W_span.T, 2)          (B*K, M, 2)
  value = head_reshape(h @ W_val.T, D)           (B*K, M, D)
  mean  = sigmoid(span0) * M ; soft = softplus(span1)
  attn  = softmax(-soft * (pos - mean)^2)        (B*K, M, M)
  out   = (attn @ value)  -> concat heads -> @ W_out.T

Shapes are hardcoded: B=2, M=2048, HS=1024, K=16 heads, D=64.

Strategy (8 NeuronCores, SPMD - one program, per-core data):
  * batch*head sharding: each core handles one batch and 4 head instances,
    assigned by a host-side greedy clustering that minimizes the shared
    windowed-schedule width per program slot.
  * Host computes the tiny span projection, sorts each head's query rows by
    their Gaussian mean, and builds a per-slot windowed schedule: for each
    128-wide key block only the contiguous range of sorted rows whose
    Gaussian window (tail exp(-9)) touches it is processed.
  * Scores are two concurrent rank-6 fp16 matmuls per 128-key block (the two
    64-wide halves packed into PE row/col groups (0,0) and (32,64)); the
    quadratic -s(u-t)^2 is expanded against a per-64-block-centered basis
    [u^2,u,1] with hi/lo-split fp16 coefficients, so scores are exact to
    ~3e-3 in f32 PSUM.  One ScalarE exp(-x) per 512-wide PSUM chunk.
  * attn @ value accumulates out^T (65 x 1024) in PSUM per row-half with the
    fp16 value tile stationary; a ones-column yields the softmax denominator.
  * Value is computed per key block (stationary hT block, moving fp16 W_val
    slice) and pipelined against the hT DMA, which streams in 256-column
    chunks; the attention strips interleave with the value matmuls.
  * The sorted->natural un-permutation runs on GPSIMD local_scatter per
    head-pair/dest-half, overlapped with later strips and the output
    projection, which is interleaved per natural half.
"""

import sys
import types

import numpy as np
import ml_dtypes

B, M, HS, NH, D = 2, 2048, 1024, 16, 64
NCORES = 8
HPC = 4            # head slots per core
CP = HPC * D       # 256 channels per core
NJB = M // 128     # 128-wide key blocks
HM = M // 2        # row half
TAIL_T = 9.0       # window cut: dropped tail mass ~ exp(-9) ~ 1.2e-4
ALIGN = 4

_CACHE = {}


def _ensure_ntff_hook():
    """Install the antenv.axon_hooks shim if the image lacks it (profiling only)."""
    try:
        import antenv.axon_hooks  # noqa: F401
        return
    except ImportError:
        pass
    try:
        import antenv
        from trn_agent_boot.trn_boot import _ntff_profile_via_ctypes
    except ImportError:
        return
    mod = types.ModuleType("antenv.axon_hooks")
    _h = [None]
    mod.set_axon_ntff_profile_hook = lambda hk: _h.__setitem__(0, hk)
    mod.get_axon_ntff_profile_hook = lambda: _h[0]
    sys.modules["antenv.axon_hooks"] = mod
    antenv.axon_hooks = mod
    try:
        mod.set_axon_ntff_profile_hook(
            _ntff_profile_via_ctypes("/opt/axon/libaxon_pjrt.so"))
    except Exception:
        pass


def _sigmoid64(x):
    return 1.0 / (1.0 + np.exp(-x.astype(np.float64)))


def _softplus64(x):
    return np.logaddexp(0.0, x.astype(np.float64))


def _band(ms, ws):
    """Per 128-block [ilo, ihi) over sorted rows whose window touches it."""
    lo, hi = ms - ws, ms + ws
    ilos = np.full(NJB, M, np.int64)
    ihis = np.zeros(NJB, np.int64)
    for jb in range(NJB):
        mask = (hi >= jb * 128) & (lo <= jb * 128 + 128)
        idx = np.flatnonzero(mask)
        if idx.size:
            ilos[jb] = idx[0]
            ihis[jb] = idx[-1] + 1
    return ilos, ihis


def _assign_slots(bands):
    """Greedy: assign instances (b,k) to 4 slots (4 per batch each),
    minimizing the summed envelope width."""
    insts = [(b, k) for b in range(B) for k in range(NH)]
    width = {bk: int((bands[bk][1] - np.minimum(bands[bk][0], bands[bk][1])).sum())
             for bk in insts}
    insts.sort(key=lambda bk: -width[bk])
    slot_lo = [np.full(NJB, M, np.int64) for _ in range(HPC)]
    slot_hi = [np.zeros(NJB, np.int64) for _ in range(HPC)]
    slot_cnt = [[0, 0] for _ in range(HPC)]
    assign = {}

    def cost(lo, hi):
        return int(np.maximum(hi - lo, 0).sum())

    for bk in insts:
        ilo, ihi = bands[bk]
        best, bestd = None, None
        for s in range(HPC):
            if slot_cnt[s][bk[0]] >= B * 2:
                continue
            nlo = np.minimum(slot_lo[s], ilo)
            nhi = np.maximum(slot_hi[s], ihi)
            d = cost(nlo, nhi) - cost(slot_lo[s], slot_hi[s])
            if bestd is None or d < bestd:
                best, bestd = s, d
        s = best
        slot_lo[s] = np.minimum(slot_lo[s], ilo)
        slot_hi[s] = np.maximum(slot_hi[s], ihi)
        slot_cnt[s][bk[0]] += 1
        assign[bk] = s
    return assign, slot_lo, slot_hi


def _build_sched(slot_lo, slot_hi):
    """Per-slot, per-half segment/chunk schedule."""
    sched = []
    for s in range(HPC):
        ranges = []
        for jb in range(NJB):
            lo, hi = int(slot_lo[s][jb]), int(slot_hi[s][jb])
            if hi <= lo:
                ranges.append((0, 0))
            else:
                ranges.append((lo & ~(ALIGN - 1),
                               min(M, (hi + ALIGN - 1) & ~(ALIGN - 1))))
        halves = []
        for uh in range(2):
            h_lo = uh * HM
            segs = []
            off = 0
            for jb in range(NJB):
                lo, hi = ranges[jb]
                s0, s1 = max(lo, h_lo), min(hi, h_lo + HM)
                if s1 <= s0:
                    continue
                segs.append((jb, s0, s1, off))
                off += s1 - s0
            cw = off
            # split segs at packed-512 and (s-h_lo)%512 boundaries
            pieces = []
            for jb, s0, s1, o0 in segs:
                cur = s0
                while cur < s1:
                    o = o0 + (cur - s0)
                    nxt = min(s1,
                              cur + (512 - (o % 512)),
                              h_lo + ((cur - h_lo) // 512 + 1) * 512)
                    pieces.append((jb, cur, nxt, o))
                    cur = nxt
            nchunks = (cw + 511) // 512
            chunks = []
            for ci in range(nchunks):
                c0, c1 = ci * 512, min(cw, (ci + 1) * 512)
                ps = [p for p in pieces if c0 <= p[3] < c1]
                need_jb = max(p[0] for p in ps)
                chunks.append({"c0": c0, "c1": c1, "pieces": ps,
                               "need_jb": need_jb})
            # last piece per o_ps bank (for stop flag)
            lastp = {}
            for i, p in enumerate(pieces):
                lastp[(p[1] - h_lo) // 512] = i
            halves.append({"cw": cw, "segs": segs, "pieces": pieces,
                           "chunks": chunks,
                           "last_by_bank": set(lastp.values())})
        sched.append({"ranges": tuple(ranges), "halves": halves})
    return sched


def _f16_split(x):
    hi = x.astype(np.float16)
    lo = (x - hi.astype(np.float64)).astype(np.float16)
    return hi, lo


def _build_host_data(h, W_span, W_val, W_out):
    h = np.asarray(h, np.float32)
    W_span = np.asarray(W_span, np.float32)
    W_val = np.asarray(W_val, np.float32)
    W_out = np.asarray(W_out, np.float32)

    span = (h.reshape(B * M, HS) @ W_span.T).reshape(B, M, 2 * NH)

    m_all = np.zeros((B, NH, M), np.float64)
    s_all = np.zeros((B, NH, M), np.float64)
    for b in range(B):
        for k in range(NH):
            m_all[b, k] = _sigmoid64(span[b, :, 2 * k]) * M
            s_all[b, k] = _softplus64(span[b, :, 2 * k + 1])
    order_all = np.argsort(m_all, axis=-1, kind="stable")
    W_all = np.sqrt(TAIL_T / np.maximum(s_all, 1e-12))

    bands = {}
    for b in range(B):
        for k in range(NH):
            o = order_all[b, k]
            bands[(b, k)] = _band(m_all[b, k][o], W_all[b, k][o])
    assign, slot_lo, slot_hi = _assign_slots(bands)
    sched = _build_sched(slot_lo, slot_hi)

    # coverage: every sorted row must fall in the range of its own mean block
    for (b, k), s in assign.items():
        ranges = sched[s]["ranges"]
        ms = m_all[b, k][order_all[b, k]]
        own = np.clip((ms // 128).astype(np.int64), 0, NJB - 1)
        pos = np.arange(M)
        lows = np.array([ranges[j][0] for j in own])
        highs = np.array([ranges[j][1] for j in own])
        if not ((lows <= pos) & (pos < highs)).all():
            raise AssertionError("window schedule does not cover all rows")

    # core (b, g) takes 4 instances of batch b, one per slot
    per_slot_heads = [[[], []] for _ in range(HPC)]
    for (b, k), s in assign.items():
        per_slot_heads[s][b].append(k)

    cwmax = max(sched[s]["halves"][uh]["cw"] for s in range(HPC)
                for uh in range(2))
    cwmax = (cwmax + 7) & ~7

    u = np.arange(-32, 32, dtype=np.float64)
    u2 = (u * u).astype(np.float16).astype(np.float64)
    basis = np.zeros((38, 64), np.float16)
    for base in (0, 32):
        basis[base + 0] = u2
        basis[base + 1] = u
        basis[base + 2] = 1.0
        basis[base + 3] = u2
        basis[base + 4] = u
        basis[base + 5] = 1.0

    in_maps = []
    for core in range(NCORES):
        b, g = core // HPC, core % HPC
        heads = [per_slot_heads[s][b][g] for s in range(HPC)]

        hT = np.ascontiguousarray(
            h[b].T.reshape(8, 128, NJB, 128).transpose(1, 2, 0, 3)
        ).astype(np.float16)
        chans = np.concatenate([np.arange(k * D, (k + 1) * D) for k in heads])
        Wv = np.ascontiguousarray(
            W_val[chans, :].T.reshape(8, 128, CP).transpose(1, 0, 2)
        ).astype(np.float16)
        Wo = np.ascontiguousarray(
            W_out[:, chans].T.reshape(2, 128, HS).transpose(1, 0, 2)
        ).astype(np.float16)

        A6 = np.zeros((HPC, 2, 12, cwmax), np.float16)
        sidx = np.zeros((128, 4, M), np.int16)
        for kk, k in enumerate(heads):
            o = order_all[b, k]
            ms = m_all[b, k][o]
            ss = s_all[b, k][o]
            for uh in range(2):
                for jb, s0, s1, off in sched[kk]["halves"][uh]["segs"]:
                    mseg, sseg = ms[s0:s1], ss[s0:s1]
                    n = s1 - s0
                    for par, center in ((0, 128 * jb + 32), (1, 128 * jb + 96)):
                        t = mseg - center
                        s_ = sseg.copy()
                        c1 = -2.0 * sseg * t
                        c0 = sseg * t * t
                        # rows far outside this 64-block: flat huge score
                        # (weight exp(-x) == 0 either way; avoids fp16 overflow)
                        far = c0 > 50000.0
                        s_[far] = 0.0
                        c1[far] = 0.0
                        c0[far] = 50000.0
                        sh, sl = _f16_split(s_)
                        c1h, c1l = _f16_split(c1)
                        c0h, c0l = _f16_split(c0)
                        rows = A6[kk, uh, 6 * par:6 * par + 6, off:off + n]
                        rows[0], rows[1], rows[2] = sh, c1h, c0h
                        rows[3], rows[4], rows[5] = sl, c1l, c0l
            p, sub = kk // 2, kk % 2
            o64 = o.astype(np.int64)
            for hh in range(2):
                arr = np.where((o64 >= hh * HM) & (o64 < (hh + 1) * HM),
                               o64 - hh * HM, -1).astype(np.int16)
                sidx[64 * sub:64 * sub + 64, 2 * p + hh, :] = arr[None, :]

        in_maps.append({
            "hT": hT, "Wv": Wv, "Wo": Wo, "A6": A6,
            "sidx": sidx, "basis": basis,
        })

    key = tuple(sched[s]["ranges"] for s in range(HPC)) + (cwmax,)
    return in_maps, key, sched, cwmax


def _build_kernel(sched, cwmax):
    import concourse.tile as tile
    from concourse import bacc, mybir
    from concourse.alu_op_type import AluOpType

    F32 = mybir.dt.float32
    F16 = mybir.dt.float16
    I16 = mybir.dt.int16

    nc = bacc.Bacc("TRN2", target_bir_lowering=False, debug=False,
                   num_devices=NCORES)

    hT = nc.dram_tensor("hT", [128, NJB, 8, 128], F16, kind="ExternalInput")
    Wv = nc.dram_tensor("Wv", [128, 8, CP], F16, kind="ExternalInput")
    Wo = nc.dram_tensor("Wo", [128, 2, HS], F16, kind="ExternalInput")
    A6 = nc.dram_tensor("A6", [HPC, 2, 12, cwmax], F16, kind="ExternalInput")
    sidx = nc.dram_tensor("sidx", [128, 4, M], I16, kind="ExternalInput")
    basis = nc.dram_tensor("basis", [38, 64], F16, kind="ExternalInput")
    out_part = nc.dram_tensor("out_part", [M, HS], F16, kind="ExternalOutput")

    with tile.TileContext(nc) as tc:
        with (
            tc.tile_pool(name="persist", bufs=1) as persist,
            tc.tile_pool(name="at_pool", bufs=6) as at_pool,
            tc.tile_pool(name="norm_pool", bufs=4) as norm_pool,
            tc.tile_pool(name="out_pool", bufs=3) as out_pool,
            tc.tile_pool(name="ps", bufs=2, space="PSUM") as ps,
        ):
            # ---- persistent tiles ----
            basis_sb = persist.tile([38, 64], F16, name="basis")
            hT_sb = persist.tile([128, NJB, 8, 128], F16, name="hT")
            Wv_sb = persist.tile([128, 8, CP], F16, name="Wv")
            Wo_sb = persist.tile([128, 2, HS], F16, name="Wo")
            sidx_sb = persist.tile([128, 4, M], I16, name="sidx")
            A6_sb = [[persist.tile([38, max(sched[kk]["halves"][uh]["cw"], 8)],
                                   F16, name=f"A6_{kk}_{uh}")
                      for uh in range(2)] for kk in range(HPC)]
            v_sb = [persist.tile([128, HPC, D + 1], F16, name=f"v{jb}")
                    for jb in range(NJB)]
            pair_sb = [persist.tile([128, M], F16, name=f"pair{p}")
                       for p in range(2)]
            nat_sb = [persist.tile([128, M], F16, name=f"nat{p}")
                      for p in range(2)]
            ones_sb = persist.tile([1, 64], F16, name="ones64")
            actw_sb = persist.tile([1, 16], F32, name="actw")
            actw_o = persist.tile([1, 16], F16, name="actwo")

            # ---- activation table preload (scalar queue head) ----
            nc.vector.memset(actw_sb[:], 1.0)
            nc.scalar.activation(actw_o[:], actw_sb[:],
                                 mybir.ActivationFunctionType.Exp, scale=-1.0)

            # ---- input DMA: hT/small on sync, A6 strips on idle gpsimd ----
            def dma_a6(kk, uh):
                cw = sched[kk]["halves"][uh]["cw"]
                if cw == 0:
                    return
                eng = nc.sync if kk == 0 else nc.gpsimd
                eng.dma_start(A6_sb[kk][uh][0:6, :cw], A6[kk, uh, 0:6, :cw])
                eng.dma_start(A6_sb[kk][uh][32:38, :cw], A6[kk, uh, 6:12, :cw])

            nc.sync.dma_start(basis_sb[:], basis[:])
            nc.sync.dma_start(hT_sb[:, 0:2], hT[:, 0:2])
            nc.sync.dma_start(Wv_sb[:], Wv[:])
            nc.sync.dma_start(hT_sb[:, 2:4], hT[:, 2:4])
            dma_a6(0, 0)
            nc.sync.dma_start(hT_sb[:, 4:6], hT[:, 4:6])
            dma_a6(0, 1)
            for kk in range(1, HPC):
                for uh in range(2):
                    dma_a6(kk, uh)
            for jc in range(3, 8):
                nc.sync.dma_start(hT_sb[:, 2 * jc:2 * jc + 2],
                                  hT[:, 2 * jc:2 * jc + 2])
            nc.sync.dma_start(sidx_sb[:], sidx[:])
            nc.sync.dma_start(Wo_sb[:], Wo[:])
            nc.vector.memset(ones_sb[:], 1.0)

            # ---- PE warmup: release the HAM throttle during initial DMA ----
            warm = ps.tile([64, 64], F32, name="warm", tag="pv", bufs=1)

            def emit_warm(n):
                for _ in range(n):
                    nc.tensor.matmul(warm[:], basis_sb[0:6, :],
                                     basis_sb[0:6, :],
                                     start=True, stop=True,
                                     tile_position=(0, 0))

            emit_warm(90)

            # ---- value per key block ----
            def emit_value(jb):
                pv = ps.tile([128, HPC, D], F32, name="pv", tag="pv", bufs=1)
                for c in range(8):
                    nc.tensor.matmul(
                        pv[:], hT_sb[:, jb, c, :], Wv_sb[:, c, :],
                        start=(c == 0), stop=(c == 7))
                nc.vector.tensor_copy(v_sb[jb][:, :, 0:D], pv[:])
                nc.vector.memset(v_sb[jb][:, :, D:D + 1], 1.0)

            # ---- attention strip cursor ----
            class Strip:
                def __init__(self, kk, uh):
                    self.kk, self.uh = kk, uh
                    self.H = sched[kk]["halves"][uh]
                    self.h_lo = uh * HM
                    self.A6t = A6_sb[kk][uh]
                    self.o_ps = ps.tile([65, HM], F32, name="oT", tag="oT",
                                        bufs=2)
                    self.bank_first = [True, True]
                    self.pend = []
                    self.ci = 0

                @property
                def done(self):
                    return self.ci >= len(self.H["chunks"]) and not self.pend

                def emit_chunk(self):
                    ch = self.H["chunks"][self.ci]
                    self.ci += 1
                    w = ch["c1"] - ch["c0"]
                    sc = ps.tile([128, 512], F32, name="sc", tag="sc", bufs=3)
                    for jb, s0, s1, off in ch["pieces"]:
                        r0 = off - ch["c0"]
                        n = s1 - s0
                        nc.tensor.matmul(
                            sc[0:64, r0:r0 + n], basis_sb[0:6, :],
                            self.A6t[0:6, off:off + n],
                            start=True, stop=True, tile_position=(0, 0))
                        nc.tensor.matmul(
                            sc[64:128, r0:r0 + n], basis_sb[32:38, :],
                            self.A6t[32:38, off:off + n],
                            start=True, stop=True, tile_position=(32, 64))
                    at_t = at_pool.tile([128, 512], F16, name="at", tag="at")
                    nc.scalar.activation(at_t[:, :w], sc[:, :w],
                                         mybir.ActivationFunctionType.Exp,
                                         scale=-1.0)
                    self.pend.append((at_t, ch))

                def flush_one(self):
                    at_t, ch = self.pend.pop(0)
                    for jb, s0, s1, off in ch["pieces"]:
                        pi = self.H["pieces"].index((jb, s0, s1, off))
                        q = (s0 - self.h_lo) // 512
                        nc.tensor.matmul(
                            self.o_ps[:, s0 - self.h_lo:s1 - self.h_lo],
                            v_sb[jb][:, self.kk, :],
                            at_t[:, off - ch["c0"]:
                                 off - ch["c0"] + (s1 - s0)],
                            start=self.bank_first[q],
                            stop=(pi in self.H["last_by_bank"]))
                        self.bank_first[q] = False

            # ---- normalization of a finished strip ----
            def emit_norm(st):
                p, sub = st.kk // 2, st.kk % 2
                h_lo = st.h_lo
                for q in range(2):
                    qs = slice(q * 512, (q + 1) * 512)
                    rcr = norm_pool.tile([1, 512], F16, name="rcr", tag="rcr")
                    nc.vector.tensor_copy(rcr[:], st.o_ps[64:65, qs])
                    bc = ps.tile([64, 512], F32, name="bc", tag="sc", bufs=3)
                    nc.tensor.matmul(bc[:], ones_sb[:], rcr[:],
                                     start=True, stop=True)
                    rcs = norm_pool.tile([64, 512], F32, name="rcs", tag="rcs")
                    nc.vector.reciprocal_approx_fast(rcs[:], bc[:])
                    nc.vector.tensor_tensor(
                        pair_sb[p][64 * sub:64 * sub + 64,
                                   h_lo + q * 512:h_lo + (q + 1) * 512],
                        st.o_ps[0:64, qs], rcs[:], AluOpType.mult)

            def scatter_pair(p):
                for hh in range(2):
                    nc.gpsimd.local_scatter(
                        nat_sb[p][:, hh * HM:(hh + 1) * HM],
                        pair_sb[p][:], sidx_sb[:, 2 * p + hh, :],
                        channels=128, num_elems=HM, num_idxs=M)

            normed = set()

            def do_norms(strips):
                for st in strips:
                    emit_norm(st)
                    normed.add((st.kk, st.uh))
                if (1, 1) in normed and (1, 0) in normed and \
                        (0, 0) in normed and (0, 1) in normed and \
                        "p0" not in normed:
                    scatter_pair(0)
                    normed.add("p0")

            # ---- phase A: value interleaved with strips (0,0) and (0,1) ----
            live = [Strip(0, 0), Strip(0, 1)]
            for jc in range(8):
                emit_value(2 * jc)
                emit_value(2 * jc + 1)
                progressed = True
                while progressed:
                    progressed = False
                    for st in live:
                        if (st.ci < len(st.H["chunks"]) and
                                st.H["chunks"][st.ci]["need_jb"] <= 2 * jc + 1):
                            st.emit_chunk()
                            progressed = True
                        if len(st.pend) > 2:
                            st.flush_one()

            # ---- phase B: rolling window of 2 live strips (FIFO retire) ----
            todo = [(1, 0), (1, 1), (2, 0), (2, 1), (3, 0), (3, 1)]
            while live or todo:
                while len(live) < 2 and todo:
                    live.append(Strip(*todo.pop(0)))
                for st in live:
                    if st.ci < len(st.H["chunks"]):
                        st.emit_chunk()
                    if st.pend and (len(st.pend) > 1 or
                                    st.ci >= len(st.H["chunks"])):
                        st.flush_one()
                while live and live[0].done:
                    do_norms([live[0]])
                    live.pop(0)
            scatter_pair(1)

            # keep PE warm across the scatter wait
            emit_warm(100)

            # ---- output projection, interleaved per natural half ----
            for hh in range(2):
                for ic in range(hh * 8, hh * 8 + 8):
                    ics = slice(ic * 128, (ic + 1) * 128)
                    ot = out_pool.tile([128, HS], F16, name="ot", tag="ot")
                    for jh in range(2):
                        jhs = slice(jh * 512, (jh + 1) * 512)
                        pp = ps.tile([128, 512], F32, name="pp", tag="sc",
                                     bufs=3)
                        nc.tensor.matmul(pp[:], nat_sb[0][:, ics],
                                         Wo_sb[:, 0, jhs],
                                         start=True, stop=False)
                        nc.tensor.matmul(pp[:], nat_sb[1][:, ics],
                                         Wo_sb[:, 1, jhs],
                                         start=False, stop=True)
                        if jh == 0:
                            nc.vector.tensor_copy(ot[:, jhs], pp[:])
                        else:
                            nc.scalar.copy(ot[:, jhs], pp[:])
                    nc.sync.dma_start(out_part[ics, :], ot[:])

    nc.compile()
    return nc


def kernel(h, W_span, W_val, W_out):
    _ensure_ntff_hook()
    from concourse.bass_utils import run_bass_kernel_spmd

    in_maps, key, sched, cwmax = _build_host_data(h, W_span, W_val, W_out)
    nc = _CACHE.get(key)
    if nc is None:
        nc = _build_kernel(sched, cwmax)
        _CACHE[key] = nc

    res = run_bass_kernel_spmd(nc, in_maps, list(range(NCORES)), trace=False)

    out = np.zeros((B, M, HS), np.float32)
    for core in range(NCORES):
        out[core // HPC] += res.results[core]["out_part"].astype(np.float32)
    return out


# revision 18
# speedup vs baseline: 1.0650x; 1.0650x over previous
"""Trainium2 Bass kernel for the Gaussian-span multi-head self-attention module.

  span  = head_reshape(h @ W_span.T, 2)          (B*K, M, 2)
  value = head_reshape(h @ W_val.T, D)           (B*K, M, D)
  mean  = sigmoid(span0) * M ; soft = softplus(span1)
  attn  = softmax(-soft * (pos - mean)^2)        (B*K, M, M)
  out   = (attn @ value)  -> concat heads -> @ W_out.T

Shapes are hardcoded: B=2, M=2048, HS=1024, K=16 heads, D=64.

Strategy (8 NeuronCores, SPMD - one program, per-core data):
  * batch*head sharding: each core handles one batch and 4 head instances,
    assigned by a host-side greedy clustering that minimizes the shared
    windowed-schedule width per program slot.
  * Host computes the tiny span projection, sorts each head's query rows by
    their Gaussian mean, and builds a per-slot windowed schedule: for each
    128-wide key block only the contiguous range of sorted rows whose
    Gaussian window (tail exp(-9)) touches it is processed.
  * Scores are two concurrent rank-6 fp16 matmuls per 128-key block (the two
    64-wide halves packed into PE row/col groups (0,0) and (32,64)); the
    quadratic -s(u-t)^2 is expanded against a per-64-block-centered basis
    [u^2,u,1] with hi/lo-split fp16 coefficients, so scores are exact to
    ~3e-3 in f32 PSUM.  One ScalarE exp(-x) per 512-wide PSUM chunk.
  * attn @ value accumulates out^T (65 x 1024) in PSUM per row-half with the
    fp16 value tile stationary; a ones-column yields the softmax denominator.
  * Value is computed per key block (stationary hT block, moving fp16 W_val
    slice) and pipelined against the hT DMA, which streams in 256-column
    chunks; the attention strips interleave with the value matmuls.
  * The sorted->natural un-permutation runs on GPSIMD local_scatter per
    head-pair/dest-half, overlapped with later strips and the output
    projection, which is interleaved per natural half.
"""

import sys
import types

import numpy as np
import ml_dtypes

B, M, HS, NH, D = 2, 2048, 1024, 16, 64
NCORES = 8
HPC = 4            # head slots per core
CP = HPC * D       # 256 channels per core
NJB = M // 128     # 128-wide key blocks
HM = M // 2        # row half
TAIL_T = 9.0       # window cut: dropped tail mass ~ exp(-9) ~ 1.2e-4
ALIGN = 4

_CACHE = {}


def _ensure_ntff_hook():
    """Install the antenv.axon_hooks shim if the image lacks it (profiling only)."""
    try:
        import antenv.axon_hooks  # noqa: F401
        return
    except ImportError:
        pass
    try:
        import antenv
        from trn_agent_boot.trn_boot import _ntff_profile_via_ctypes
    except ImportError:
        return
    mod = types.ModuleType("antenv.axon_hooks")
    _h = [None]
    mod.set_axon_ntff_profile_hook = lambda hk: _h.__setitem__(0, hk)
    mod.get_axon_ntff_profile_hook = lambda: _h[0]
    sys.modules["antenv.axon_hooks"] = mod
    antenv.axon_hooks = mod
    try:
        mod.set_axon_ntff_profile_hook(
            _ntff_profile_via_ctypes("/opt/axon/libaxon_pjrt.so"))
    except Exception:
        pass


def _sigmoid64(x):
    return 1.0 / (1.0 + np.exp(-x.astype(np.float64)))


def _softplus64(x):
    return np.logaddexp(0.0, x.astype(np.float64))


def _band(ms, ws):
    """Per 128-block [ilo, ihi) over sorted rows whose window touches it."""
    lo, hi = ms - ws, ms + ws
    ilos = np.full(NJB, M, np.int64)
    ihis = np.zeros(NJB, np.int64)
    for jb in range(NJB):
        mask = (hi >= jb * 128) & (lo <= jb * 128 + 128)
        idx = np.flatnonzero(mask)
        if idx.size:
            ilos[jb] = idx[0]
            ihis[jb] = idx[-1] + 1
    return ilos, ihis


def _assign_slots(bands):
    """Greedy: assign instances (b,k) to 4 slots (4 per batch each),
    minimizing the summed envelope width."""
    insts = [(b, k) for b in range(B) for k in range(NH)]
    width = {bk: int((bands[bk][1] - np.minimum(bands[bk][0], bands[bk][1])).sum())
             for bk in insts}
    insts.sort(key=lambda bk: -width[bk])
    slot_lo = [np.full(NJB, M, np.int64) for _ in range(HPC)]
    slot_hi = [np.zeros(NJB, np.int64) for _ in range(HPC)]
    slot_cnt = [[0, 0] for _ in range(HPC)]
    assign = {}

    def cost(lo, hi):
        return int(np.maximum(hi - lo, 0).sum())

    for bk in insts:
        ilo, ihi = bands[bk]
        best, bestd = None, None
        for s in range(HPC):
            if slot_cnt[s][bk[0]] >= B * 2:
                continue
            nlo = np.minimum(slot_lo[s], ilo)
            nhi = np.maximum(slot_hi[s], ihi)
            d = cost(nlo, nhi) - cost(slot_lo[s], slot_hi[s])
            if bestd is None or d < bestd:
                best, bestd = s, d
        s = best
        slot_lo[s] = np.minimum(slot_lo[s], ilo)
        slot_hi[s] = np.maximum(slot_hi[s], ihi)
        slot_cnt[s][bk[0]] += 1
        assign[bk] = s
    return assign, slot_lo, slot_hi


def _build_sched(slot_lo, slot_hi):
    """Per-slot, per-half segment/chunk schedule."""
    sched = []
    for s in range(HPC):
        ranges = []
        for jb in range(NJB):
            lo, hi = int(slot_lo[s][jb]), int(slot_hi[s][jb])
            if hi <= lo:
                ranges.append((0, 0))
            else:
                ranges.append((lo & ~(ALIGN - 1),
                               min(M, (hi + ALIGN - 1) & ~(ALIGN - 1))))
        halves = []
        for uh in range(2):
            h_lo = uh * HM
            segs = []
            off = 0
            for jb in range(NJB):
                lo, hi = ranges[jb]
                s0, s1 = max(lo, h_lo), min(hi, h_lo + HM)
                if s1 <= s0:
                    continue
                segs.append((jb, s0, s1, off))
                off += s1 - s0
            cw = off
            # split segs at packed-512 and (s-h_lo)%512 boundaries
            pieces = []
            for jb, s0, s1, o0 in segs:
                cur = s0
                while cur < s1:
                    o = o0 + (cur - s0)
                    nxt = min(s1,
                              cur + (512 - (o % 512)),
                              h_lo + ((cur - h_lo) // 512 + 1) * 512)
                    pieces.append((jb, cur, nxt, o))
                    cur = nxt
            nchunks = (cw + 511) // 512
            chunks = []
            for ci in range(nchunks):
                c0, c1 = ci * 512, min(cw, (ci + 1) * 512)
                ps = [p for p in pieces if c0 <= p[3] < c1]
                need_jb = max(p[0] for p in ps)
                chunks.append({"c0": c0, "c1": c1, "pieces": ps,
                               "need_jb": need_jb})
            # last piece per o_ps bank (for stop flag)
            lastp = {}
            for i, p in enumerate(pieces):
                lastp[(p[1] - h_lo) // 512] = i
            halves.append({"cw": cw, "segs": segs, "pieces": pieces,
                           "chunks": chunks,
                           "last_by_bank": set(lastp.values())})
        sched.append({"ranges": tuple(ranges), "halves": halves})
    return sched


def _f16_split(x):
    hi = x.astype(np.float16)
    lo = (x - hi.astype(np.float64)).astype(np.float16)
    return hi, lo


def _build_host_data(h, W_span, W_val, W_out):
    h = np.asarray(h, np.float32)
    W_span = np.asarray(W_span, np.float32)
    W_val = np.asarray(W_val, np.float32)
    W_out = np.asarray(W_out, np.float32)

    span = (h.reshape(B * M, HS) @ W_span.T).reshape(B, M, 2 * NH)

    m_all = np.zeros((B, NH, M), np.float64)
    s_all = np.zeros((B, NH, M), np.float64)
    for b in range(B):
        for k in range(NH):
            m_all[b, k] = _sigmoid64(span[b, :, 2 * k]) * M
            s_all[b, k] = _softplus64(span[b, :, 2 * k + 1])
    order_all = np.argsort(m_all, axis=-1, kind="stable")
    W_all = np.sqrt(TAIL_T / np.maximum(s_all, 1e-12))

    bands = {}
    for b in range(B):
        for k in range(NH):
            o = order_all[b, k]
            bands[(b, k)] = _band(m_all[b, k][o], W_all[b, k][o])
    assign, slot_lo, slot_hi = _assign_slots(bands)
    sched = _build_sched(slot_lo, slot_hi)

    # coverage: every sorted row must fall in the range of its own mean block
    for (b, k), s in assign.items():
        ranges = sched[s]["ranges"]
        ms = m_all[b, k][order_all[b, k]]
        own = np.clip((ms // 128).astype(np.int64), 0, NJB - 1)
        pos = np.arange(M)
        lows = np.array([ranges[j][0] for j in own])
        highs = np.array([ranges[j][1] for j in own])
        if not ((lows <= pos) & (pos < highs)).all():
            raise AssertionError("window schedule does not cover all rows")

    # core (b, g) takes 4 instances of batch b, one per slot
    per_slot_heads = [[[], []] for _ in range(HPC)]
    for (b, k), s in assign.items():
        per_slot_heads[s][b].append(k)

    cwmax = max(sched[s]["halves"][uh]["cw"] for s in range(HPC)
                for uh in range(2))
    cwmax = (cwmax + 7) & ~7

    u = np.arange(-32, 32, dtype=np.float64)
    u2 = (u * u).astype(np.float16).astype(np.float64)
    basis = np.zeros((38, 64), np.float16)
    for base in (0, 32):
        basis[base + 0] = u2
        basis[base + 1] = u
        basis[base + 2] = 1.0
        basis[base + 3] = u2
        basis[base + 4] = u
        basis[base + 5] = 1.0

    in_maps = []
    for core in range(NCORES):
        b, g = core // HPC, core % HPC
        heads = [per_slot_heads[s][b][g] for s in range(HPC)]

        hT = np.ascontiguousarray(
            h[b].T.reshape(8, 128, NJB, 128).transpose(1, 2, 0, 3)
        ).astype(np.float16)
        chans = np.concatenate([np.arange(k * D, (k + 1) * D) for k in heads])
        Wv = np.ascontiguousarray(
            W_val[chans, :].T.reshape(8, 128, CP).transpose(1, 0, 2)
        ).astype(np.float16)
        Wo = np.ascontiguousarray(
            W_out[:, chans].T.reshape(2, 128, HS).transpose(1, 0, 2)
        ).astype(np.float16)

        A6 = np.zeros((HPC, 2, 12, cwmax), np.float16)
        sidx = np.zeros((128, 4, M), np.int16)
        for kk, k in enumerate(heads):
            o = order_all[b, k]
            ms = m_all[b, k][o]
            ss = s_all[b, k][o]
            for uh in range(2):
                for jb, s0, s1, off in sched[kk]["halves"][uh]["segs"]:
                    mseg, sseg = ms[s0:s1], ss[s0:s1]
                    n = s1 - s0
                    for par, center in ((0, 128 * jb + 32), (1, 128 * jb + 96)):
                        t = mseg - center
                        s_ = sseg.copy()
                        c1 = -2.0 * sseg * t
                        c0 = sseg * t * t
                        # rows far outside this 64-block: flat huge score
                        # (weight exp(-x) == 0 either way; avoids fp16 overflow)
                        far = c0 > 50000.0
                        s_[far] = 0.0
                        c1[far] = 0.0
                        c0[far] = 50000.0
                        sh, sl = _f16_split(s_)
                        c1h, c1l = _f16_split(c1)
                        c0h, c0l = _f16_split(c0)
                        rows = A6[kk, uh, 6 * par:6 * par + 6, off:off + n]
                        rows[0], rows[1], rows[2] = sh, c1h, c0h
                        rows[3], rows[4], rows[5] = sl, c1l, c0l
            p, sub = kk // 2, kk % 2
            o64 = o.astype(np.int64)
            for hh in range(2):
                arr = np.where((o64 >= hh * HM) & (o64 < (hh + 1) * HM),
                               o64 - hh * HM, -1).astype(np.int16)
                sidx[64 * sub:64 * sub + 64, 2 * p + hh, :] = arr[None, :]

        in_maps.append({
            "hT": hT, "Wv": Wv, "Wo": Wo, "A6": A6,
            "sidx": sidx, "basis": basis,
        })

    key = tuple(sched[s]["ranges"] for s in range(HPC)) + (cwmax,)
    return in_maps, key, sched, cwmax


def _build_kernel(sched, cwmax):
    import concourse.tile as tile
    from concourse import bacc, mybir
    from concourse.alu_op_type import AluOpType

    F32 = mybir.dt.float32
    F16 = mybir.dt.float16
    I16 = mybir.dt.int16

    nc = bacc.Bacc("TRN2", target_bir_lowering=False, debug=False,
                   num_devices=NCORES)

    hT = nc.dram_tensor("hT", [128, NJB, 8, 128], F16, kind="ExternalInput")
    Wv = nc.dram_tensor("Wv", [128, 8, CP], F16, kind="ExternalInput")
    Wo = nc.dram_tensor("Wo", [128, 2, HS], F16, kind="ExternalInput")
    A6 = nc.dram_tensor("A6", [HPC, 2, 12, cwmax], F16, kind="ExternalInput")
    sidx = nc.dram_tensor("sidx", [128, 4, M], I16, kind="ExternalInput")
    basis = nc.dram_tensor("basis", [38, 64], F16, kind="ExternalInput")
    out_part = nc.dram_tensor("out_part", [M, HS], F16, kind="ExternalOutput")

    with tile.TileContext(nc) as tc:
        with (
            tc.tile_pool(name="persist", bufs=1) as persist,
            tc.tile_pool(name="at_pool", bufs=12) as at_pool,
            tc.tile_pool(name="norm_pool", bufs=6) as norm_pool,
            tc.tile_pool(name="out_pool", bufs=3) as out_pool,
            tc.tile_pool(name="ps", bufs=2, space="PSUM") as ps,
        ):
            # ---- persistent tiles ----
            basis_sb = persist.tile([38, 64], F16, name="basis")
            hT_sb = persist.tile([128, NJB, 8, 128], F16, name="hT")
            Wv_sb = persist.tile([128, 8, CP], F16, name="Wv")
            Wo_sb = persist.tile([128, 2, HS], F16, name="Wo")
            sidx_sb = persist.tile([128, 4, M], I16, name="sidx")
            A6_sb = [[persist.tile([38, max(sched[kk]["halves"][uh]["cw"], 8)],
                                   F16, name=f"A6_{kk}_{uh}")
                      for uh in range(2)] for kk in range(HPC)]
            v_sb = [persist.tile([128, HPC, D + 1], F16, name=f"v{jb}")
                    for jb in range(NJB)]
            pair_sb = [persist.tile([128, M], F16, name=f"pair{p}")
                       for p in range(2)]
            nat_sb = [persist.tile([128, M], F16, name=f"nat{p}")
                      for p in range(2)]
            ones_sb = persist.tile([1, 64], F16, name="ones64")
            actw_sb = persist.tile([1, 16], F32, name="actw")
            actw_o = persist.tile([1, 16], F16, name="actwo")

            # ---- activation table preload (scalar queue head) ----
            nc.vector.memset(actw_sb[:], 1.0)
            nc.scalar.activation(actw_o[:], actw_sb[:],
                                 mybir.ActivationFunctionType.Exp, scale=-1.0)

            # ---- input DMA: hT/small on sync, A6 strips on idle gpsimd ----
            def dma_a6(kk, uh):
                cw = sched[kk]["halves"][uh]["cw"]
                if cw == 0:
                    return
                eng = nc.sync if kk == 0 else nc.gpsimd
                eng.dma_start(A6_sb[kk][uh][0:6, :cw], A6[kk, uh, 0:6, :cw])
                eng.dma_start(A6_sb[kk][uh][32:38, :cw], A6[kk, uh, 6:12, :cw])

            nc.sync.dma_start(basis_sb[:], basis[:])
            nc.sync.dma_start(hT_sb[:, 0:2], hT[:, 0:2])
            nc.sync.dma_start(Wv_sb[:], Wv[:])
            nc.sync.dma_start(hT_sb[:, 2:4], hT[:, 2:4])
            dma_a6(0, 0)
            nc.sync.dma_start(hT_sb[:, 4:6], hT[:, 4:6])
            dma_a6(0, 1)
            for kk in range(1, HPC):
                for uh in range(2):
                    dma_a6(kk, uh)
            for jc in range(3, 8):
                nc.sync.dma_start(hT_sb[:, 2 * jc:2 * jc + 2],
                                  hT[:, 2 * jc:2 * jc + 2])
            nc.sync.dma_start(sidx_sb[:], sidx[:])
            nc.sync.dma_start(Wo_sb[:], Wo[:])
            nc.vector.memset(ones_sb[:], 1.0)

            # ---- PE warmup: release the HAM throttle during initial DMA ----
            warm = ps.tile([64, 64], F32, name="warm", tag="pv", bufs=1)

            def emit_warm(n):
                for _ in range(n):
                    nc.tensor.matmul(warm[:], basis_sb[0:6, :],
                                     basis_sb[0:6, :],
                                     start=True, stop=True,
                                     tile_position=(0, 0))

            emit_warm(90)

            # ---- value per key block ----
            def emit_value(jb):
                pv = ps.tile([128, HPC, D], F32, name="pv", tag="pv", bufs=1)
                for c in range(8):
                    nc.tensor.matmul(
                        pv[:], hT_sb[:, jb, c, :], Wv_sb[:, c, :],
                        start=(c == 0), stop=(c == 7))
                nc.vector.tensor_copy(v_sb[jb][:, :, 0:D], pv[:])
                nc.vector.memset(v_sb[jb][:, :, D:D + 1], 1.0)

            # ---- attention strip cursor ----
            class Strip:
                def __init__(self, kk, uh):
                    self.kk, self.uh = kk, uh
                    self.H = sched[kk]["halves"][uh]
                    self.h_lo = uh * HM
                    self.A6t = A6_sb[kk][uh]
                    self.o_ps = ps.tile([65, HM], F32, name="oT", tag="oT",
                                        bufs=2)
                    self.bank_first = [True, True]
                    self.pend = []
                    self.ci = 0

                @property
                def done(self):
                    return self.ci >= len(self.H["chunks"]) and not self.pend

                def emit_chunk(self):
                    ch = self.H["chunks"][self.ci]
                    self.ci += 1
                    w = ch["c1"] - ch["c0"]
                    sc = ps.tile([128, 512], F32, name="sc", tag="sc", bufs=3)
                    for jb, s0, s1, off in ch["pieces"]:
                        r0 = off - ch["c0"]
                        n = s1 - s0
                        nc.tensor.matmul(
                            sc[0:64, r0:r0 + n], basis_sb[0:6, :],
                            self.A6t[0:6, off:off + n],
                            start=True, stop=True, tile_position=(0, 0))
                        nc.tensor.matmul(
                            sc[64:128, r0:r0 + n], basis_sb[32:38, :],
                            self.A6t[32:38, off:off + n],
                            start=True, stop=True, tile_position=(32, 64))
                    at_t = at_pool.tile([128, 512], F16, name="at", tag="at")
                    nc.scalar.activation(at_t[:, :w], sc[:, :w],
                                         mybir.ActivationFunctionType.Exp,
                                         scale=-1.0)
                    self.pend.append((at_t, ch))

                def flush_one(self):
                    at_t, ch = self.pend.pop(0)
                    for jb, s0, s1, off in ch["pieces"]:
                        pi = self.H["pieces"].index((jb, s0, s1, off))
                        q = (s0 - self.h_lo) // 512
                        nc.tensor.matmul(
                            self.o_ps[:, s0 - self.h_lo:s1 - self.h_lo],
                            v_sb[jb][:, self.kk, :],
                            at_t[:, off - ch["c0"]:
                                 off - ch["c0"] + (s1 - s0)],
                            start=self.bank_first[q],
                            stop=(pi in self.H["last_by_bank"]))
                        self.bank_first[q] = False

            # ---- normalization of a finished strip ----
            def emit_norm(st):
                p, sub = st.kk // 2, st.kk % 2
                h_lo = st.h_lo
                for q in range(2):
                    qs = slice(q * 512, (q + 1) * 512)
                    rcr = norm_pool.tile([1, 512], F16, name="rcr", tag="rcr")
                    nc.vector.tensor_copy(rcr[:], st.o_ps[64:65, qs])
                    bc = ps.tile([64, 512], F32, name="bc", tag="pv", bufs=1)
                    nc.tensor.matmul(bc[:], ones_sb[:], rcr[:],
                                     start=True, stop=True)
                    rcs = norm_pool.tile([64, 512], F32, name="rcs", tag="rcs")
                    nc.vector.reciprocal_approx_fast(rcs[:], bc[:])
                    nc.vector.tensor_tensor(
                        pair_sb[p][64 * sub:64 * sub + 64,
                                   h_lo + q * 512:h_lo + (q + 1) * 512],
                        st.o_ps[0:64, qs], rcs[:], AluOpType.mult)

            def scatter_pair(p):
                for hh in range(2):
                    nc.gpsimd.local_scatter(
                        nat_sb[p][:, hh * HM:(hh + 1) * HM],
                        pair_sb[p][:], sidx_sb[:, 2 * p + hh, :],
                        channels=128, num_elems=HM, num_idxs=M)

            normed = set()

            def do_norms(strips):
                for st in strips:
                    emit_norm(st)
                    normed.add((st.kk, st.uh))
                if (1, 1) in normed and (1, 0) in normed and \
                        (0, 0) in normed and (0, 1) in normed and \
                        "p0" not in normed:
                    scatter_pair(0)
                    normed.add("p0")

            # ---- phase A: value interleaved with strips (0,0) and (0,1) ----
            live = [Strip(0, 0), Strip(0, 1)]
            for jc in range(8):
                emit_value(2 * jc)
                emit_value(2 * jc + 1)
                progressed = True
                while progressed:
                    progressed = False
                    for st in live:
                        if (st.ci < len(st.H["chunks"]) and
                                st.H["chunks"][st.ci]["need_jb"] <= 2 * jc + 1):
                            st.emit_chunk()
                            progressed = True
                        if len(st.pend) > 2:
                            st.flush_one()

            # ---- phase B: rolling window of 2 live strips (FIFO retire) ----
            todo = [(1, 0), (1, 1), (2, 0), (2, 1), (3, 0), (3, 1)]
            while live or todo:
                while len(live) < 2 and todo:
                    live.append(Strip(*todo.pop(0)))
                for st in live:
                    if st.ci < len(st.H["chunks"]):
                        st.emit_chunk()
                    if st.pend and (len(st.pend) > 1 or
                                    st.ci >= len(st.H["chunks"])):
                        st.flush_one()
                while live and live[0].done:
                    do_norms([live[0]])
                    live.pop(0)
            scatter_pair(1)

            # keep PE warm across the scatter wait
            emit_warm(100)

            # ---- output projection, interleaved per natural half ----
            for hh in range(2):
                for ic in range(hh * 8, hh * 8 + 8):
                    ics = slice(ic * 128, (ic + 1) * 128)
                    ot = out_pool.tile([128, HS], F16, name="ot", tag="ot")
                    for jh in range(2):
                        jhs = slice(jh * 512, (jh + 1) * 512)
                        pp = ps.tile([128, 512], F32, name="pp", tag="sc",
                                     bufs=3)
                        nc.tensor.matmul(pp[:], nat_sb[0][:, ics],
                                         Wo_sb[:, 0, jhs],
                                         start=True, stop=False)
                        nc.tensor.matmul(pp[:], nat_sb[1][:, ics],
                                         Wo_sb[:, 1, jhs],
                                         start=False, stop=True)
                        if jh == 0:
                            nc.vector.tensor_copy(ot[:, jhs], pp[:])
                        else:
                            nc.scalar.copy(ot[:, jhs], pp[:])
                    nc.sync.dma_start(out_part[ics, :], ot[:])

    nc.compile()
    return nc


def kernel(h, W_span, W_val, W_out):
    _ensure_ntff_hook()
    from concourse.bass_utils import run_bass_kernel_spmd

    in_maps, key, sched, cwmax = _build_host_data(h, W_span, W_val, W_out)
    nc = _CACHE.get(key)
    if nc is None:
        nc = _build_kernel(sched, cwmax)
        _CACHE[key] = nc

    res = run_bass_kernel_spmd(nc, in_maps, list(range(NCORES)), trace=False)

    out = np.zeros((B, M, HS), np.float32)
    for core in range(NCORES):
        out[core // HPC] += res.results[core]["out_part"].astype(np.float32)
    return out
